# revision 40
# baseline (speedup 1.0000x reference)
"""AttentionWithMSR Trainium2 kernel — 8-core SPMD, data-parallel over (batch, H-half).

Self-contained: takes FULL inputs, shards internally, returns FULL output.

Math (reference):
    msr  = log1p(x) - (1/3) * sum_s log1p(blur_s(x)),  s in {15, 80, 250}
    a    = BN(conv1x1(g;  Wg)),  b = BN(conv1x1(msr; Wx))
    psi  = sigmoid(BN(conv1x1(relu(a + b); wpsi)))
    out  = x * psi

Kernel mapping:
  * blur_s(x) per (b, c) image M as two 256x256 matmuls: blur = G_s @ M @ G_s
    (G_s symmetric Toeplitz from the zero-padded normalized 1D Gaussian).
    Each core owns one batch sample b and one 128-row H-half:
      pass A (vertical, transposed): Vt = M^T @ GvT    [w(256, 2 chunks), h_own(128)]
      pass B (horizontal):           Blur = Vt^T @ G_s [h_own(128), w(256)]
    Both passes run fp8-e4m3 DoubleRow matmuls (K=256 folded per instruction);
    G operands are pre-scaled x64 into fp8 range and descaled for free by the
    Ln activation's input scale (1/4096).
  * BN folded into conv weights on host. Ln is pair-batched into a
    scale-major quad tile; l12 = l15+l80 is one contiguous quad DVE add and
    ships with l3 (straight from the Ln output) through one fp8 DRAM round
    trip into the Ko=1 half of the conv rhs.
  * The a+b conv runs as two fp8 K=128 accumulation groups per pixel block:
    w1 = [Wg; Wx] on prefetched [g; log1p(x)] (issued early to bridge the
    gather latency) + w2 = [-Wx/3; -Wx/3] on [l12; l3]. Weights are x8 in
    fp8; the Relu activation descales via its input scale (1/8) and adds the
    folded bias per partition.
  * psi conv replicates its scalar output over 64 partitions inside the
    matmul (replicated-weight lhsT) so sigmoid/multiply run full-width.
  * Ring discipline: scalar/vector never issue DMA triggers during phase 1
    (a trigger blocked on ring credit stalls the engine FIFO); prefetches and
    intermediates ride the sync/gpsimd rings.
"""

import sys

sys.path.insert(0, "/opt/trn_rl_repo")

import numpy as np
import ml_dtypes

SCALES = (15, 80, 250)
EPS = 1e-5
B, C, H, W = 4, 64, 256, 256
HALF = 128
FINT = 32
N_CORES = 8
BF16 = ml_dtypes.bfloat16
F8 = ml_dtypes.float8_e4m3
NPIX = HALF * W  # 32768 pixels per core
NPIX2 = NPIX // 2

_CACHE = {}
_LAST_IN_MAPS = None


def _gauss_mat(scale: int) -> np.ndarray:
    """256x256 matrix of the zero-padded 'same' normalized 1D Gaussian blur."""
    k = int(4 * scale + 1)
    p = k // 2
    coords = np.arange(k, dtype=np.float32) - (k - 1) / 2.0
    g1 = np.exp(-(coords**2) / np.float32(2.0 * scale * scale))
    g1 = g1 / g1.sum()
    i = np.arange(W)
    D = i[None, :] - i[:, None]  # j - i
    M = np.where(np.abs(D) <= p, g1[np.clip(D + p, 0, k - 1)], np.float32(0.0))
    return M.astype(np.float32)


def _build_nc():
    import concourse.mybir as mybir
    import concourse.tile as tile
    from concourse import bacc

    bf = mybir.dt.bfloat16
    f8 = mybir.dt.float8e4
    f32 = mybir.dt.float32
    AF = mybir.ActivationFunctionType
    DR = mybir.MatmulPerfMode.DoubleRow

    nc = bacc.Bacc("TRN2", target_bir_lowering=False)

    # x[b] transposed to (h-chunk, h-in-chunk, channel, w); chunk 0 = own half.
    xbt_e = nc.dram_tensor("xbt", [2, HALF, C, W], f8, kind="ExternalInput")
    gb_e = nc.dram_tensor("gb", [C, NPIX], f8, kind="ExternalInput")
    lxh_e = nc.dram_tensor("lxh", [C, NPIX], f8, kind="ExternalInput")
    xh2_e = nc.dram_tensor("xh2", [2, 64, NPIX2], bf, kind="ExternalInput")
    gvt_e = nc.dram_tensor("gvt", [128, 2, 384], f8, kind="ExternalInput")
    gh_e = nc.dram_tensor("gh", [128, 3, 2, W], f8, kind="ExternalInput")
    wc_e = nc.dram_tensor("wc", [128, 2, FINT], f8, kind="ExternalInput")
    wpsi_e = nc.dram_tensor("wpsi", [128, 64], bf, kind="ExternalInput")
    bias0_e = nc.dram_tensor("bias0", [128, 1], f32, kind="ExternalInput")
    bpsi_e = nc.dram_tensor("bpsi", [128, 1], f32, kind="ExternalInput")
    out_e = nc.dram_tensor("out", [128, NPIX2], bf, kind="ExternalOutput")

    with tile.TileContext(nc) as tc:
        with (
            tc.tile_pool(name="consts", bufs=1) as consts,
            tc.tile_pool(name="dram", bufs=1, space="DRAM") as dpool,
        ):
            gvt_sb = consts.tile([128, 2, 384], f8)
            gh_sb = consts.tile([128, 3, 2, W], f8)
            wc_sb = consts.tile([128, 2, FINT], f8)
            wpsi_sb = consts.tile([128, 64], bf)
            bias0_sb = consts.tile([128, 1], f32)
            bpsi_sb = consts.tile([128, 1], f32)
            # combined conv rhs: Ko=0 -> [g; lxh] (prefetched),
            # Ko=1 -> [l12; l3] (gathered from the phase-1 round trip).
            # Two pixel-half tiles keep the Ko stride under the 16-bit
            # ISA step field (32768 would overflow).
            rhsC0 = consts.tile([128, 2, NPIX2], f8)
            rhsC1 = consts.tile([128, 2, NPIX2], f8)
            xh_sb = consts.tile([128, NPIX2], bf)  # multiply operand
            nc.sync.dma_start(gvt_sb[:], gvt_e[:])

            md1 = dpool.tile([C, NPIX], f8)
            md3 = dpool.tile([C, NPIX], f8)

            # ---- phase 1: blur + s123 combine, software-pipelined over channels
            with (
                tc.tile_pool(name="p1", bufs=4) as p1,
                tc.tile_pool(name="p1b", bufs=4) as p1b,
                tc.tile_pool(name="p1x", bufs=3) as p1x,
                tc.tile_pool(name="p1vt", bufs=2, space="PSUM") as p1vt,
                tc.tile_pool(name="p1bl", bufs=1, space="PSUM") as p1bl,
            ):
                GRP = 8  # channels per staged load
                xs_tiles = {}
                vt_tiles = {}

                H2 = GRP // 2

                def load_group(g):
                    xs_g = p1x.tile([128, 2, GRP, W], f8, tag="xs")
                    if g == 0:
                        nc.scalar.dma_start(xs_g[:, 0, 0:H2], xbt_e[0, :, 0:H2, :])
                        nc.gpsimd.dma_start(xs_g[:, 1, 0:H2], xbt_e[1, :, 0:H2, :])
                        nc.sync.dma_start(
                            xs_g[:, 0, H2:GRP], xbt_e[0, :, H2:GRP, :]
                        )
                        nc.gpsimd.dma_start(
                            xs_g[:, 1, H2:GRP], xbt_e[1, :, H2:GRP, :]
                        )
                    else:
                        nc.gpsimd.dma_start(
                            xs_g[:, 0], xbt_e[0, :, g * GRP : (g + 1) * GRP, :]
                        )
                        nc.gpsimd.dma_start(
                            xs_g[:, 1], xbt_e[1, :, g * GRP : (g + 1) * GRP, :]
                        )
                    xs_tiles[g] = xs_g

                load_group(0)

                nc.sync.dma_start(gh_sb[:], gh_e[:])
                nc.sync.dma_start(wc_sb[:], wc_e[:])
                nc.sync.dma_start(wpsi_sb[:], wpsi_e[:])
                nc.sync.dma_start(bias0_sb[:], bias0_e[:])
                nc.sync.dma_start(bpsi_sb[:], bpsi_e[:])

                # g/lx/x prefetch chunks, all on the sync ring (no compute
                # engine ever blocks behind a waiting DMA trigger).
                prefetch = []
                for q in range(8):
                    sl = slice(q * 4096, (q + 1) * 4096)
                    rq = (rhsC0, rhsC1)[q // 4]
                    sq = slice((q % 4) * 4096, (q % 4 + 1) * 4096)
                    prefetch.append((nc.sync, rq[0:64, 0, sq], gb_e[:, sl]))
                    prefetch.append((nc.sync, rq[64:128, 0, sq], lxh_e[:, sl]))
                for q in range(4):
                    sl = slice(q * 4096, (q + 1) * 4096)
                    prefetch.append((nc.sync, xh_sb[0:64, sl], xh2_e[0, :, sl]))
                    prefetch.append(
                        (nc.sync, xh_sb[64:128, sl], xh2_e[1, :, sl])
                    )

                def pass_a(c):
                    xs_g = xs_tiles[c // GRP]
                    ci = c % GRP
                    vt_ps = p1vt.tile([128, 1024], f32, tag="vtps")
                    for wc in range(2):
                        nc.tensor.matmul(
                            vt_ps[:, wc * 512 : wc * 512 + 384],
                            lhsT=xs_g[:, :, ci, wc * 128 : (wc + 1) * 128],
                            rhs=gvt_sb[:],
                            start=True,
                            stop=True,
                            perf_mode=DR,
                        )
                    vt_sb = p1.tile([128, 2, 384], f8, tag="vt")
                    vt_v = vt_ps[:].rearrange("p (a q) -> p a q", q=512)
                    nc.vector.tensor_copy(vt_sb[:], vt_v[:, :, 0:384])
                    vt_tiles[c] = vt_sb

                md1_v = md1[:].rearrange("c (h w) -> h c w", h=HALF)
                md3_v = md3[:].rearrange("c (h w) -> h c w", h=HALF)
                blq_tiles = {}
                lf_quads = {}

                def pass_b(c):
                    vt_sb = vt_tiles.pop(c)
                    if c % 2 == 0:
                        blq_new = p1bl.tile([128, 2, 768], f32, tag="blp")
                        blq_tiles[c // 2] = blq_new
                    blur_ps = blq_tiles[c // 2]
                    for s in range(3):
                        nc.tensor.matmul(
                            blur_ps[:, c % 2, s * 256 : (s + 1) * 256],
                            lhsT=vt_sb[:, :, s * 128 : (s + 1) * 128],
                            rhs=gh_sb[:, s],
                            start=True,
                            stop=True,
                            perf_mode=DR,
                        )
                    # per-channel Ln into a scale-major QUAD tile: releases
                    # each blur-pair half a channel earlier (slice-level dep)
                    if c % 4 == 0:
                        lf_new = p1b.tile([128, 3, 4, 256], f8, tag="lf")
                        lf_quads[c // 4] = lf_new
                    l_f = lf_quads[c // 4]
                    nc.scalar.activation(
                        l_f[:, :, c % 4, :],
                        blur_ps[:, c % 2, :].rearrange("p (s w) -> p s w", s=3),
                        AF.Ln,
                        bias=1.0,
                        scale=1.0 / 4096.0,
                    )
                    if c % 2 == 1:
                        blq_tiles.pop(c // 2)
                    if c % 4 != 3:
                        return
                    l_f = lf_quads.pop(c // 4)
                    l12q = p1b.tile([128, 4, 256], f8, tag="s12")
                    nc.gpsimd.tensor_add(l12q[:], l_f[:, 0], l_f[:, 1])
                    nc.sync.dma_start(md1_v[:, c - 3 : c + 1, :], l12q[:])
                    nc.gpsimd.dma_start(md3_v[:, c - 3 : c + 1, :], l_f[:, 2])

                load_group(1)
                pass_a(0)
                for c in range(C):
                    if c % 2 == 0 and c // 2 < len(prefetch):
                        eng, dst, src_ap = prefetch[c // 2]
                        eng.dma_start(dst, src_ap)
                    if (c + 1) % GRP == 0 and (c + 1) // GRP + 1 < C // GRP:
                        load_group((c + 1) // GRP + 1)
                    if c + 1 < C:
                        pass_a(c + 1)
                    pass_b(c)

                # Dummy tail: hold the PE clock gate open across the
                # phase-1 -> phase-2 PSUM pool turnover.
                for w in range(3):
                    warm_ps = p1bl.tile([128, 2, 768], f32, tag="blp")
                    for s in range(3):
                        nc.tensor.matmul(
                            warm_ps[:, 0, s * 256 : (s + 1) * 256],
                            lhsT=gh_sb[:, 0, :, 0:128],
                            rhs=gh_sb[:, 1],
                            start=True,
                            stop=True,
                            perf_mode=DR,
                        )

            # ---- phase 2: conv1x1 + relu + psi + sigmoid + multiply.
            # rhs1 ([g; log1p(x)]) is fully prefetched, so the w1 matmuls
            # bridge the PE across the s123 gather latency.
            with (
                tc.tile_pool(name="p2", bufs=2) as p2,
                tc.tile_pool(name="p2ab", bufs=4, space="PSUM") as p2ab,
                tc.tile_pool(name="p2s", bufs=2, space="PSUM") as p2s,
            ):
                ab_tiles = {}

                rings = [nc.sync, nc.gpsimd]
                orings = [nc.sync, nc.gpsimd, nc.scalar]
                NBRIDGE = 4  # w1 groups pre-issued to bridge the gather wait

                def gather_rhs2(grp):
                    # all gathers ride sync, in grp order: deep pipeline with
                    # no out-write interleaving on the same ring
                    px = grp * 2048
                    rq = (rhsC0, rhsC1)[grp // 8]
                    lpx = px % NPIX2
                    nc.sync.dma_start(
                        rq[0:64, 1, lpx : lpx + 2048], md1[:, px : px + 2048]
                    )
                    nc.sync.dma_start(
                        rq[64:128, 1, lpx : lpx + 2048], md3[:, px : px + 2048]
                    )

                def conv_w1(grp):
                    # bridge mode: non-DR fp8 matmul on the prefetched half
                    lpx = (grp * 2048) % NPIX2
                    rq = (rhsC0, rhsC1)[grp // 8]
                    ab_ps = p2ab.tile([128, 512], f32, tag="abps")
                    for t in range(4):
                        nc.tensor.matmul(
                            ab_ps[32 * t : 32 * t + 32, :],
                            lhsT=wc_sb[:, 0, :],
                            rhs=rq[:, 0, lpx + 512 * t : lpx + 512 * (t + 1)],
                            start=True,
                            stop=False,
                            tile_position=(0, 32 * t),
                        )
                    ab_tiles[grp] = ab_ps

                for grp in range(NBRIDGE):
                    gather_rhs2(grp)
                    conv_w1(grp)
                for grp in range(NBRIDGE, min(NBRIDGE + 2, 16)):
                    gather_rhs2(grp)
                # (w1 for later groups issued inside the main loop)
                
                for grp in range(16):
                    lpx = (grp * 2048) % NPIX2
                    rq = (rhsC0, rhsC1)[grp // 8]
                    ab_ps = ab_tiles.pop(grp)
                    for t in range(4):
                        nc.tensor.matmul(
                            ab_ps[32 * t : 32 * t + 32, :],
                            lhsT=wc_sb[:, 1, :],
                            rhs=rq[:, 1, lpx + 512 * t : lpx + 512 * (t + 1)],
                            start=False,
                            stop=True,
                            tile_position=(0, 32 * t),
                        )
                    if grp + 3 < 16:
                        gather_rhs2(grp + 3)
                    if grp + NBRIDGE < 16:
                        conv_w1(grp + NBRIDGE)
                    relu_sb = p2.tile([128, 512], bf, tag="relu")
                    nc.scalar.activation(
                        relu_sb[:],
                        ab_ps[:],
                        AF.Relu,
                        bias=bias0_sb[:],
                        scale=1.0 / 8.0,
                    )
                    s_ps = p2s.tile([128, 1024], f32, tag="sps")
                    for t in range(4):
                        a, bb = t // 2, t % 2
                        nc.tensor.matmul(
                            s_ps[64 * a : 64 * a + 64, 512 * bb : 512 * bb + 512],
                            lhsT=wpsi_sb[32 * t : 32 * t + 32, :],
                            rhs=relu_sb[32 * t : 32 * t + 32, :],
                            start=True,
                            stop=True,
                            tile_position=(32 * t, 64 * a),
                        )
                    psi_sb = p2.tile([128, 1024], bf, tag="psi")
                    nc.scalar.activation(
                        psi_sb[:], s_ps[:], AF.Sigmoid, bias=bpsi_sb[:]
                    )
                    out2 = p2.tile([128, 1024], bf, tag="out2")
                    nc.vector.tensor_mul(
                        out2[:], xh_sb[:, grp * 1024 : (grp + 1) * 1024], psi_sb[:]
                    )
                    if grp >= 14:
                        nc.gpsimd.dma_start(
                            out_e[:, grp * 1024 : grp * 1024 + 512], out2[:, 0:512]
                        )
                        nc.sync.dma_start(
                            out_e[:, grp * 1024 + 512 : (grp + 1) * 1024],
                            out2[:, 512:1024],
                        )
                    else:
                        rings[(grp + 1) % 2].dma_start(
                            out_e[:, grp * 1024 : (grp + 1) * 1024], out2[:]
                        )

    nc.finalize()
    return nc


def kernel(**inputs):
    from concourse.bass_utils import run_bass_kernel_spmd

    g = np.asarray(inputs["g"], dtype=np.float32)
    x = np.asarray(inputs["x"], dtype=np.float32)

    def f(name):
        return np.asarray(inputs[name], dtype=np.float32)

    # Fold eval-mode BN into the 1x1 convs.
    ag = f("wg_gamma") / np.sqrt(f("wg_var") + EPS)
    wg_eff = ag[:, None] * f("wg_w")[:, :, 0, 0]  # [32, 64]
    bg_eff = ag * (f("wg_b") - f("wg_mean")) + f("wg_beta")
    ax = f("wx_gamma") / np.sqrt(f("wx_var") + EPS)
    wx_eff = ax[:, None] * f("wx_w")[:, :, 0, 0]  # [32, 64]
    bx_eff = ax * (f("wx_b") - f("wx_mean")) + f("wx_beta")
    ap_ = f("psi_gamma") / np.sqrt(f("psi_var") + EPS)
    wpsi_eff = ap_[0] * f("psi_w")[0, :, 0, 0]  # [32]
    bpsi = float(ap_[0] * (f("psi_b")[0] - f("psi_mean")[0]) + f("psi_beta")[0])
    bias0 = bg_eff + bx_eff  # [32]

    Gs = [_gauss_mat(s) for s in SCALES]

    # combined conv weights (x8, descaled in the relu activation):
    # Ko=0 -> [Wg; Wx] against [g; lxh], Ko=1 -> [-Wx/3; -Wx/3] against [l12; l3]
    wc_t = np.empty((128, 2, FINT), dtype=np.float32)
    wc_t[:, 0, :] = np.concatenate([wg_eff.T, wx_eff.T], axis=0) * 8.0
    wc_t[:, 1, :] = np.concatenate([-wx_eff.T / 3.0, -wx_eff.T / 3.0], axis=0) * 8.0
    wc_t = wc_t.astype(F8)
    wpsi_t = np.broadcast_to(
        np.tile(wpsi_eff, 4)[:, None], (128, 64)
    ).astype(BF16)  # [128, 64]: row 32t+o = wpsi[o], replicated over 64 cols
    bias0_t = np.tile(bias0, 4)[:, None].astype(np.float32)  # [128, 1]
    bpsi_t = np.full((128, 1), bpsi, dtype=np.float32)

    # gh[wi, s, wc, w] = 64 * G_s[wc*128+wi, w]   (pass-B DoubleRow moving op)
    gh = np.empty((128, 3, 2, W), dtype=np.float32)
    for s in range(3):
        for wc in range(2):
            gh[:, s, wc, :] = Gs[s][wc * 128 : (wc + 1) * 128, :] * 64.0
    gh = gh.astype(F8)

    key = "nc"
    if key not in _CACHE:
        _CACHE[key] = _build_nc()
    nc = _CACHE[key]

    in_maps = []
    for core in range(N_CORES):
        b, half = core // 2, core % 2
        h0 = half * HALF
        # gvt[hp, slot, s*128+ho] = G_s[chunk*128+hp, h0+ho], chunk = slot ^ half
        gvt = np.empty((128, 2, 384), dtype=np.float32)
        for slot in range(2):
            chunk = slot ^ half
            for s in range(3):
                gvt[:, slot, s * 128 : (s + 1) * 128] = (
                    Gs[s][chunk * 128 : (chunk + 1) * 128, h0 : h0 + HALF]
                    * 64.0
                )
        xr = x[b].reshape(C, 2, HALF, W).transpose(1, 2, 0, 3)  # [2, HALF, C, W]
        if half == 1:
            xr = xr[::-1]
        xown = x[b, :, h0 : h0 + HALF, :].reshape(C, NPIX)
        xrr = xown.reshape(C, 16, 2, 1024)
        xh2 = np.stack(
            [xrr[:, :, 0].reshape(C, NPIX2), xrr[:, :, 1].reshape(C, NPIX2)]
        )

        in_maps.append(
            {
                "xbt": np.ascontiguousarray(xr).astype(F8),
                "gb": g[b, :, h0 : h0 + HALF, :].reshape(C, NPIX).astype(F8),
                "lxh": np.log1p(x[b, :, h0 : h0 + HALF, :])
                .reshape(C, NPIX)
                .astype(F8),
                "xh2": xh2.astype(BF16),
                "gvt": gvt.astype(F8),
                "gh": gh,
                "wc": wc_t,
                "wpsi": wpsi_t,
                "bias0": bias0_t,
                "bpsi": bpsi_t,
            }
        )

    global _LAST_IN_MAPS
    _LAST_IN_MAPS = in_maps
    res = run_bass_kernel_spmd(nc, in_maps, core_ids=list(range(N_CORES)))

    out = np.empty((B, C, H, W), dtype=np.float32)
    for core in range(N_CORES):
        b, half = core // 2, core % 2
        h0 = half * HALF
        r = np.asarray(res.results[core]["out"]).reshape(2, C, 16, 1024)
        o = np.empty((C, 16, 2, 1024), dtype=np.float32)
        o[:, :, 0] = r[0]
        o[:, :, 1] = r[1]
        out[b, :, h0 : h0 + HALF, :] = o.reshape(C, HALF, W)
    return out


# revision 41
# speedup vs baseline: 1.1449x; 1.1449x over previous
"""AttentionWithMSR Trainium2 kernel — 8-core SPMD, data-parallel over (batch, H-half).

Self-contained: takes FULL inputs, shards internally, returns FULL output.

Math (reference):
    msr  = log1p(x) - (1/3) * sum_s log1p(blur_s(x)),  s in {15, 80, 250}
    a    = BN(conv1x1(g;  Wg)),  b = BN(conv1x1(msr; Wx))
    psi  = sigmoid(BN(conv1x1(relu(a + b); wpsi)))
    out  = x * psi

Kernel mapping:
  * blur_s(x) per (b, c) image M as two 256x256 matmuls: blur = G_s @ M @ G_s
    (G_s symmetric Toeplitz from the zero-padded normalized 1D Gaussian).
    Each core owns one batch sample b and one 128-row H-half:
      pass A (vertical, transposed): Vt = M^T @ GvT    [w(256, 2 chunks), h_own(128)]
      pass B (horizontal):           Blur = Vt^T @ G_s [h_own(128), w(256)]
    Both passes run fp8-e4m3 DoubleRow matmuls (K=256 folded per instruction);
    G operands are pre-scaled x64 into fp8 range and descaled for free by the
    Ln activation's input scale (1/4096).
  * BN folded into conv weights on host. Ln is pair-batched into a
    scale-major quad tile; l12 = l15+l80 is one contiguous quad DVE add and
    ships with l3 (straight from the Ln output) through one fp8 DRAM round
    trip into the Ko=1 half of the conv rhs.
  * The a+b conv runs as two fp8 K=128 accumulation groups per pixel block:
    w1 = [Wg; Wx] on prefetched [g; log1p(x)] (issued early to bridge the
    gather latency) + w2 = [-Wx/3; -Wx/3] on [l12; l3]. Weights are x8 in
    fp8; the Relu activation descales via its input scale (1/8) and adds the
    folded bias per partition.
  * psi conv replicates its scalar output over 64 partitions inside the
    matmul (replicated-weight lhsT) so sigmoid/multiply run full-width.
  * Ring discipline: scalar/vector never issue DMA triggers during phase 1
    (a trigger blocked on ring credit stalls the engine FIFO); prefetches and
    intermediates ride the sync/gpsimd rings.
"""

import sys

sys.path.insert(0, "/opt/trn_rl_repo")

import numpy as np
import ml_dtypes

SCALES = (15, 80, 250)
EPS = 1e-5
B, C, H, W = 4, 64, 256, 256
HALF = 128
FINT = 32
N_CORES = 8
BF16 = ml_dtypes.bfloat16
F8 = ml_dtypes.float8_e4m3
NPIX = HALF * W  # 32768 pixels per core
NPIX2 = NPIX // 2

_CACHE = {}
_LAST_IN_MAPS = None


def _gauss_mat(scale: int) -> np.ndarray:
    """256x256 matrix of the zero-padded 'same' normalized 1D Gaussian blur."""
    k = int(4 * scale + 1)
    p = k // 2
    coords = np.arange(k, dtype=np.float32) - (k - 1) / 2.0
    g1 = np.exp(-(coords**2) / np.float32(2.0 * scale * scale))
    g1 = g1 / g1.sum()
    i = np.arange(W)
    D = i[None, :] - i[:, None]  # j - i
    M = np.where(np.abs(D) <= p, g1[np.clip(D + p, 0, k - 1)], np.float32(0.0))
    return M.astype(np.float32)


def _build_nc():
    import concourse.mybir as mybir
    import concourse.tile as tile
    from concourse import bacc

    bf = mybir.dt.bfloat16
    f8 = mybir.dt.float8e4
    f32 = mybir.dt.float32
    AF = mybir.ActivationFunctionType
    DR = mybir.MatmulPerfMode.DoubleRow

    nc = bacc.Bacc("TRN2", target_bir_lowering=False)

    # x[b] transposed to (h-chunk, h-in-chunk, channel, w); chunk 0 = own half.
    xbt_e = nc.dram_tensor("xbt", [2, HALF, C, W], f8, kind="ExternalInput")
    gb_e = nc.dram_tensor("gb", [C, NPIX], f8, kind="ExternalInput")
    lxh_e = nc.dram_tensor("lxh", [C, NPIX], f8, kind="ExternalInput")
    xh2_e = nc.dram_tensor("xh2", [2, 64, NPIX2], bf, kind="ExternalInput")
    gvt_e = nc.dram_tensor("gvt", [128, 2, 384], f8, kind="ExternalInput")
    gh_e = nc.dram_tensor("gh", [128, 3, 2, W], f8, kind="ExternalInput")
    wc_e = nc.dram_tensor("wc", [128, 2, FINT], f8, kind="ExternalInput")
    wpsi_e = nc.dram_tensor("wpsi", [128, 64], bf, kind="ExternalInput")
    bias0_e = nc.dram_tensor("bias0", [128, 1], f32, kind="ExternalInput")
    bpsi_e = nc.dram_tensor("bpsi", [128, 1], f32, kind="ExternalInput")
    out_e = nc.dram_tensor("out", [128, NPIX2], bf, kind="ExternalOutput")

    with tile.TileContext(nc) as tc:
        with (
            tc.tile_pool(name="consts", bufs=1) as consts,
            tc.tile_pool(name="dram", bufs=1, space="DRAM") as dpool,
        ):
            gvt_sb = consts.tile([128, 2, 384], f8)
            gh_sb = consts.tile([128, 3, 2, W], f8)
            wc_sb = consts.tile([128, 2, FINT], f8)
            wpsi_sb = consts.tile([128, 64], bf)
            bias0_sb = consts.tile([128, 1], f32)
            bpsi_sb = consts.tile([128, 1], f32)
            # combined conv rhs: Ko=0 -> [g; lxh] (prefetched),
            # Ko=1 -> [l12; l3] (gathered from the phase-1 round trip).
            # Two pixel-half tiles keep the Ko stride under the 16-bit
            # ISA step field (32768 would overflow).
            rhsC0 = consts.tile([128, 2, NPIX2], f8)
            rhsC1 = consts.tile([128, 2, NPIX2], f8)
            xh_sb = consts.tile([128, NPIX2], bf)  # multiply operand
            nc.sync.dma_start(gvt_sb[:], gvt_e[:])

            md1 = dpool.tile([C, NPIX], f8)
            md3 = dpool.tile([C, NPIX], f8)

            # ---- phase 1: blur + s123 combine, software-pipelined over channels
            with (
                tc.tile_pool(name="p1", bufs=4) as p1,
                tc.tile_pool(name="p1b", bufs=4) as p1b,
                tc.tile_pool(name="p1x", bufs=3) as p1x,
                tc.tile_pool(name="p1vt", bufs=2, space="PSUM") as p1vt,
                tc.tile_pool(name="p1bl", bufs=1, space="PSUM") as p1bl,
            ):
                GRP = 8  # channels per staged load
                xs_tiles = {}
                vt_tiles = {}

                H2 = GRP // 2

                def load_group(g):
                    xs_g = p1x.tile([128, 2, GRP, W], f8, tag="xs")
                    if g == 0:
                        nc.scalar.dma_start(xs_g[:, 0, 0:H2], xbt_e[0, :, 0:H2, :])
                        nc.gpsimd.dma_start(xs_g[:, 1, 0:H2], xbt_e[1, :, 0:H2, :])
                        nc.sync.dma_start(
                            xs_g[:, 0, H2:GRP], xbt_e[0, :, H2:GRP, :]
                        )
                        nc.gpsimd.dma_start(
                            xs_g[:, 1, H2:GRP], xbt_e[1, :, H2:GRP, :]
                        )
                    else:
                        nc.gpsimd.dma_start(
                            xs_g[:, 0], xbt_e[0, :, g * GRP : (g + 1) * GRP, :]
                        )
                        nc.gpsimd.dma_start(
                            xs_g[:, 1], xbt_e[1, :, g * GRP : (g + 1) * GRP, :]
                        )
                    xs_tiles[g] = xs_g

                load_group(0)

                nc.sync.dma_start(gh_sb[:], gh_e[:])
                nc.sync.dma_start(wc_sb[:], wc_e[:])
                nc.sync.dma_start(wpsi_sb[:], wpsi_e[:])
                nc.sync.dma_start(bias0_sb[:], bias0_e[:])
                nc.sync.dma_start(bpsi_sb[:], bpsi_e[:])

                # g/lx/x prefetch chunks, all on the sync ring (no compute
                # engine ever blocks behind a waiting DMA trigger).
                prefetch = []
                for q in range(8):
                    sl = slice(q * 4096, (q + 1) * 4096)
                    rq = (rhsC0, rhsC1)[q // 4]
                    sq = slice((q % 4) * 4096, (q % 4 + 1) * 4096)
                    prefetch.append((nc.sync, rq[0:64, 0, sq], gb_e[:, sl]))
                    prefetch.append((nc.sync, rq[64:128, 0, sq], lxh_e[:, sl]))
                for q in range(4):
                    sl = slice(q * 4096, (q + 1) * 4096)
                    prefetch.append((nc.sync, xh_sb[0:64, sl], xh2_e[0, :, sl]))
                    prefetch.append(
                        (nc.sync, xh_sb[64:128, sl], xh2_e[1, :, sl])
                    )

                def pass_a(c):
                    xs_g = xs_tiles[c // GRP]
                    ci = c % GRP
                    vt_ps = p1vt.tile([128, 1024], f32, tag="vtps")
                    for wc in range(2):
                        nc.tensor.matmul(
                            vt_ps[:, wc * 512 : wc * 512 + 384],
                            lhsT=xs_g[:, :, ci, wc * 128 : (wc + 1) * 128],
                            rhs=gvt_sb[:],
                            start=True,
                            stop=True,
                            perf_mode=DR,
                        )
                    vt_sb = p1.tile([128, 2, 384], f8, tag="vt")
                    vt_v = vt_ps[:].rearrange("p (a q) -> p a q", q=512)
                    nc.vector.tensor_copy(vt_sb[:], vt_v[:, :, 0:384])
                    vt_tiles[c] = vt_sb

                md1_v = md1[:].rearrange("c (h w) -> h c w", h=HALF)
                md3_v = md3[:].rearrange("c (h w) -> h c w", h=HALF)
                blq_tiles = {}
                lf_quads = {}

                def pass_b(c):
                    vt_sb = vt_tiles.pop(c)
                    if c % 2 == 0:
                        blq_new = p1bl.tile([128, 2, 768], f32, tag="blp")
                        blq_tiles[c // 2] = blq_new
                    blur_ps = blq_tiles[c // 2]
                    for s in range(3):
                        nc.tensor.matmul(
                            blur_ps[:, c % 2, s * 256 : (s + 1) * 256],
                            lhsT=vt_sb[:, :, s * 128 : (s + 1) * 128],
                            rhs=gh_sb[:, s],
                            start=True,
                            stop=True,
                            perf_mode=DR,
                        )
                    if c % 2 != 1:
                        return
                    # pair-Ln into a scale-major QUAD tile; at quad end one
                    # contiguous add (l12) + two DMAs (l12, l3 straight out)
                    blur_q = blq_tiles.pop(c // 2)
                    if c % 4 == 1:
                        lf_new = p1b.tile([128, 3, 4, 256], f8, tag="lf")
                        lf_quads[c // 4] = lf_new
                    l_f = lf_quads[c // 4]
                    ch2 = (c // 2) % 2
                    nc.scalar.activation(
                        l_f[:, :, 2 * ch2 : 2 * ch2 + 2, :].rearrange(
                            "p s ch w -> p ch s w"
                        ),
                        blur_q[:].rearrange("p ch (s w) -> p ch s w", s=3),
                        AF.Ln,
                        bias=1.0,
                        scale=1.0 / 4096.0,
                    )
                    if c % 4 != 3:
                        return
                    l_f = lf_quads.pop(c // 4)
                    l12q = p1b.tile([128, 4, 256], f8, tag="s12")
                    nc.gpsimd.tensor_add(l12q[:], l_f[:, 0], l_f[:, 1])
                    nc.sync.dma_start(md1_v[:, c - 3 : c + 1, :], l12q[:])
                    nc.gpsimd.dma_start(md3_v[:, c - 3 : c + 1, :], l_f[:, 2])

                load_group(1)
                pass_a(0)
                for c in range(C):
                    if c % 2 == 0 and c // 2 < len(prefetch):
                        eng, dst, src_ap = prefetch[c // 2]
                        eng.dma_start(dst, src_ap)
                    if (c + 1) % GRP == 0 and (c + 1) // GRP + 1 < C // GRP:
                        load_group((c + 1) // GRP + 1)
                    if c + 1 < C:
                        pass_a(c + 1)
                    pass_b(c)

                # Dummy tail: hold the PE clock gate open across the
                # phase-1 -> phase-2 PSUM pool turnover.
                for w in range(3):
                    warm_ps = p1bl.tile([128, 2, 768], f32, tag="blp")
                    for s in range(3):
                        nc.tensor.matmul(
                            warm_ps[:, 0, s * 256 : (s + 1) * 256],
                            lhsT=gh_sb[:, 0, :, 0:128],
                            rhs=gh_sb[:, 1],
                            start=True,
                            stop=True,
                            perf_mode=DR,
                        )

            # ---- phase 2: conv1x1 + relu + psi + sigmoid + multiply.
            # rhs1 ([g; log1p(x)]) is fully prefetched, so the w1 matmuls
            # bridge the PE across the s123 gather latency.
            with (
                tc.tile_pool(name="p2", bufs=2) as p2,
                tc.tile_pool(name="p2ab", bufs=4, space="PSUM") as p2ab,
                tc.tile_pool(name="p2s", bufs=2, space="PSUM") as p2s,
            ):
                ab_tiles = {}

                rings = [nc.sync, nc.gpsimd]
                orings = [nc.sync, nc.gpsimd, nc.scalar]
                NBRIDGE = 4  # w1 groups pre-issued to bridge the gather wait

                def gather_rhs2(grp):
                    # all gathers ride sync, in grp order: deep pipeline with
                    # no out-write interleaving on the same ring
                    px = grp * 2048
                    rq = (rhsC0, rhsC1)[grp // 8]
                    lpx = px % NPIX2
                    nc.sync.dma_start(
                        rq[0:64, 1, lpx : lpx + 2048], md1[:, px : px + 2048]
                    )
                    nc.sync.dma_start(
                        rq[64:128, 1, lpx : lpx + 2048], md3[:, px : px + 2048]
                    )

                def conv_w1(grp):
                    # bridge mode: non-DR fp8 matmul on the prefetched half
                    lpx = (grp * 2048) % NPIX2
                    rq = (rhsC0, rhsC1)[grp // 8]
                    ab_ps = p2ab.tile([128, 512], f32, tag="abps")
                    for t in range(4):
                        nc.tensor.matmul(
                            ab_ps[32 * t : 32 * t + 32, :],
                            lhsT=wc_sb[:, 0, :],
                            rhs=rq[:, 0, lpx + 512 * t : lpx + 512 * (t + 1)],
                            start=True,
                            stop=False,
                            tile_position=(0, 32 * t),
                        )
                    ab_tiles[grp] = ab_ps

                for grp in range(NBRIDGE):
                    gather_rhs2(grp)
                    conv_w1(grp)
                for grp in range(NBRIDGE, min(NBRIDGE + 2, 16)):
                    gather_rhs2(grp)
                # (w1 for later groups issued inside the main loop)
                
                for grp in range(16):
                    lpx = (grp * 2048) % NPIX2
                    rq = (rhsC0, rhsC1)[grp // 8]
                    ab_ps = ab_tiles.pop(grp)
                    for t in range(4):
                        nc.tensor.matmul(
                            ab_ps[32 * t : 32 * t + 32, :],
                            lhsT=wc_sb[:, 1, :],
                            rhs=rq[:, 1, lpx + 512 * t : lpx + 512 * (t + 1)],
                            start=False,
                            stop=True,
                            tile_position=(0, 32 * t),
                        )
                    if grp + 3 < 16:
                        gather_rhs2(grp + 3)
                    if grp + NBRIDGE < 16:
                        conv_w1(grp + NBRIDGE)
                    relu_sb = p2.tile([128, 512], bf, tag="relu")
                    nc.scalar.activation(
                        relu_sb[:],
                        ab_ps[:],
                        AF.Relu,
                        bias=bias0_sb[:],
                        scale=1.0 / 8.0,
                    )
                    s_ps = p2s.tile([128, 1024], f32, tag="sps")
                    for t in range(4):
                        a, bb = t // 2, t % 2
                        nc.tensor.matmul(
                            s_ps[64 * a : 64 * a + 64, 512 * bb : 512 * bb + 512],
                            lhsT=wpsi_sb[32 * t : 32 * t + 32, :],
                            rhs=relu_sb[32 * t : 32 * t + 32, :],
                            start=True,
                            stop=True,
                            tile_position=(32 * t, 64 * a),
                        )
                    psi_sb = p2.tile([128, 1024], bf, tag="psi")
                    nc.scalar.activation(
                        psi_sb[:], s_ps[:], AF.Sigmoid, bias=bpsi_sb[:]
                    )
                    out2 = p2.tile([128, 1024], bf, tag="out2")
                    nc.vector.tensor_mul(
                        out2[:], xh_sb[:, grp * 1024 : (grp + 1) * 1024], psi_sb[:]
                    )
                    if grp >= 14:
                        nc.gpsimd.dma_start(
                            out_e[:, grp * 1024 : grp * 1024 + 512], out2[:, 0:512]
                        )
                        nc.sync.dma_start(
                            out_e[:, grp * 1024 + 512 : (grp + 1) * 1024],
                            out2[:, 512:1024],
                        )
                    else:
                        rings[(grp + 1) % 2].dma_start(
                            out_e[:, grp * 1024 : (grp + 1) * 1024], out2[:]
                        )

    nc.finalize()
    return nc


def kernel(**inputs):
    from concourse.bass_utils import run_bass_kernel_spmd

    g = np.asarray(inputs["g"], dtype=np.float32)
    x = np.asarray(inputs["x"], dtype=np.float32)

    def f(name):
        return np.asarray(inputs[name], dtype=np.float32)

    # Fold eval-mode BN into the 1x1 convs.
    ag = f("wg_gamma") / np.sqrt(f("wg_var") + EPS)
    wg_eff = ag[:, None] * f("wg_w")[:, :, 0, 0]  # [32, 64]
    bg_eff = ag * (f("wg_b") - f("wg_mean")) + f("wg_beta")
    ax = f("wx_gamma") / np.sqrt(f("wx_var") + EPS)
    wx_eff = ax[:, None] * f("wx_w")[:, :, 0, 0]  # [32, 64]
    bx_eff = ax * (f("wx_b") - f("wx_mean")) + f("wx_beta")
    ap_ = f("psi_gamma") / np.sqrt(f("psi_var") + EPS)
    wpsi_eff = ap_[0] * f("psi_w")[0, :, 0, 0]  # [32]
    bpsi = float(ap_[0] * (f("psi_b")[0] - f("psi_mean")[0]) + f("psi_beta")[0])
    bias0 = bg_eff + bx_eff  # [32]

    Gs = [_gauss_mat(s) for s in SCALES]

    # combined conv weights (x8, descaled in the relu activation):
    # Ko=0 -> [Wg; Wx] against [g; lxh], Ko=1 -> [-Wx/3; -Wx/3] against [l12; l3]
    wc_t = np.empty((128, 2, FINT), dtype=np.float32)
    wc_t[:, 0, :] = np.concatenate([wg_eff.T, wx_eff.T], axis=0) * 8.0
    wc_t[:, 1, :] = np.concatenate([-wx_eff.T / 3.0, -wx_eff.T / 3.0], axis=0) * 8.0
    wc_t = wc_t.astype(F8)
    wpsi_t = np.broadcast_to(
        np.tile(wpsi_eff, 4)[:, None], (128, 64)
    ).astype(BF16)  # [128, 64]: row 32t+o = wpsi[o], replicated over 64 cols
    bias0_t = np.tile(bias0, 4)[:, None].astype(np.float32)  # [128, 1]
    bpsi_t = np.full((128, 1), bpsi, dtype=np.float32)

    # gh[wi, s, wc, w] = 64 * G_s[wc*128+wi, w]   (pass-B DoubleRow moving op)
    gh = np.empty((128, 3, 2, W), dtype=np.float32)
    for s in range(3):
        for wc in range(2):
            gh[:, s, wc, :] = Gs[s][wc * 128 : (wc + 1) * 128, :] * 64.0
    gh = gh.astype(F8)

    key = "nc"
    if key not in _CACHE:
        _CACHE[key] = _build_nc()
    nc = _CACHE[key]

    in_maps = []
    for core in range(N_CORES):
        b, half = core // 2, core % 2
        h0 = half * HALF
        # gvt[hp, slot, s*128+ho] = G_s[chunk*128+hp, h0+ho], chunk = slot ^ half
        gvt = np.empty((128, 2, 384), dtype=np.float32)
        for slot in range(2):
            chunk = slot ^ half
            for s in range(3):
                gvt[:, slot, s * 128 : (s + 1) * 128] = (
                    Gs[s][chunk * 128 : (chunk + 1) * 128, h0 : h0 + HALF]
                    * 64.0
                )
        xr = x[b].reshape(C, 2, HALF, W).transpose(1, 2, 0, 3)  # [2, HALF, C, W]
        if half == 1:
            xr = xr[::-1]
        xown = x[b, :, h0 : h0 + HALF, :].reshape(C, NPIX)
        xrr = xown.reshape(C, 16, 2, 1024)
        xh2 = np.stack(
            [xrr[:, :, 0].reshape(C, NPIX2), xrr[:, :, 1].reshape(C, NPIX2)]
        )

        in_maps.append(
            {
                "xbt": np.ascontiguousarray(xr).astype(F8),
                "gb": g[b, :, h0 : h0 + HALF, :].reshape(C, NPIX).astype(F8),
                "lxh": np.log1p(x[b, :, h0 : h0 + HALF, :])
                .reshape(C, NPIX)
                .astype(F8),
                "xh2": xh2.astype(BF16),
                "gvt": gvt.astype(F8),
                "gh": gh,
                "wc": wc_t,
                "wpsi": wpsi_t,
                "bias0": bias0_t,
                "bpsi": bpsi_t,
            }
        )

    global _LAST_IN_MAPS
    _LAST_IN_MAPS = in_maps
    res = run_bass_kernel_spmd(nc, in_maps, core_ids=list(range(N_CORES)))

    out = np.empty((B, C, H, W), dtype=np.float32)
    for core in range(N_CORES):
        b, half = core // 2, core % 2
        h0 = half * HALF
        r = np.asarray(res.results[core]["out"]).reshape(2, C, 16, 1024)
        o = np.empty((C, 16, 2, 1024), dtype=np.float32)
        o[:, :, 0] = r[0]
        o[:, :, 1] = r[1]
        out[b, :, h0 : h0 + HALF, :] = o.reshape(C, HALF, W)
    return out
